# revision 3
# baseline (speedup 1.0000x reference)
"""Trainium2 Bass kernel for nn_HEMoETorch_43722767073393 (moe_routing), v4.

Same math as v2 (see kernel_v2.py docstring): host computes
h_merged = relu(embed[x] @ w1^T) in fp32 (the RBF slow branch underflows to
exactly 0 in fp32), ships h_merged^T bf16; the device runs only the
vocab-sharded logits matmul  out^T = W2_shard @ hm^T  per core.

v5 device-side changes, each HW-measured:
  - w2 streamed in 1 MB group DMAs (4 vocab blocks each, 3-deep
    prefetch) instead of 256 KB per vocab block: HW-measured -130 us.
    In-flight DMA-writes into SBUF interfere with PE streaming; fewer,
    bigger transfers reduce that.
  - hm^T loaded as 8 full-row 1 MB DMAs: chunked variants (16x512KB or
    64x128KB, with n-outer early-start matmul ordering) measured
    ~40-75 us SLOWER overall despite overlapping the load tail.
  - (LDWEIGHTS-count reduction was also tried: deleting redundant
    ldweights breaks numerics - walrus pairs each matmul with its own
    ldweights - and timing showed ldweights count doesn't matter.
    Out-DMA issue-ring choice (SP/ACT/GPSIMD HWDGE/SWDGE) is neutral.)
  - hm^T loaded in 16 half-row chunks, n-major, so the first vocab
    blocks (ordered n-outer) start while the tail is in flight.
  - PSUM fp32 -> SBUF fp16 eviction split across DVE and ACT; output DMA
    as fp16 (halves out-bytes; host upcasts - logits max ~2 so fp16
    rounding is ~1e-4 relative).
"""

import numpy as np
import ml_dtypes

import concourse.bass as bass  # noqa: F401  (bass must import before bacc)
import concourse.mybir as mybir
import concourse.tile as tile
from concourse import bacc
from concourse.bass_utils import run_bass_kernel_spmd

BF16 = ml_dtypes.bfloat16

N_CORES = 8
B, S = 4, 1024
N = B * S            # 4096 tokens
D = 1024
V = 50257
VS = 6283            # ceil(V / 8); padded total = 50264
JT = D // 128        # 8 contraction tiles
NCH = N // 512       # 8 token chunks (one PSUM bank each)
VB = 50              # ceil(VS/128): 128-wide vocab blocks (padded to 6400)
VG = (VB + 3) // 4   # 13 groups of 4 vocab blocks (1 MB DMA each)
SIGMA = 2.0
FAST_RATIO = 0.7
TOP_K = 8

_prog_cache: dict = {}


def _ap_sig(arg):
    try:
        mname = tuple(m.name for m in arg.memorylocations)
    except Exception:
        mname = getattr(arg, "name", None)
    return (mname, arg.offset, str(getattr(arg, "pattern", None)),
            str(getattr(arg, "dtype", None)))


def _thin_ldweights(nc, keep=2):
    """Delete InstLdweights beyond the first `keep` in each run of
    consecutive identical weight loads on the PE (both HW weight buffers
    hold the weights after two loads)."""
    n_del = 0
    PE = mybir.EngineType.PE
    for blk in nc.main_func.blocks:
        last_sig, run, kept = None, 0, []
        for inst in blk.instructions:
            if isinstance(inst, mybir.InstLdweights):
                sig = _ap_sig(inst.ins[0])
                si = inst.sync_info
                clean = si is None or (len(si.on_wait) == 0
                                       and len(si.on_update) == 0)
                if sig == last_sig and clean:
                    run += 1
                    if run >= keep:
                        n_del += 1
                        continue
                else:
                    run = 0
                last_sig = sig
            elif isinstance(inst, mybir.InstMatmult):
                pass  # consumes weights, does not modify buffers
            elif getattr(inst, "engine", None) == PE and not isinstance(
                    inst, mybir.InstEventSemaphore):
                last_sig, run = None, 0
            kept.append(inst)
        blk.instructions[:] = kept
    return n_del


def build_program(N=N, D=D, VB=VB, num_devices=N_CORES, reps: int = 1):
    """Per-core SPMD program: out^T = W2_shard @ hm^T (bf16, fp16 out)."""
    JT = D // 128
    VG = (VB + 3) // 4
    nc = bacc.Bacc("TRN2", target_bir_lowering=False, debug=False,
                   num_devices=num_devices)
    bf = mybir.dt.bfloat16
    f16 = mybir.dt.float16
    f32 = mybir.dt.float32

    hmt_d = nc.dram_tensor("ht", [D, N], bf, kind="ExternalInput").ap()
    # pre-grouped on host: w2p4[g*128+p, b*1024 + j*128 + c]
    #   = w2T[j*128+p, (4g+b)*128 + c]
    w2p_d = nc.dram_tensor("w2p", [VG * 128, 4096], bf,
                           kind="ExternalInput").ap()
    out_d = nc.dram_tensor("out", [VB * 128, N], f16, kind="ExternalOutput").ap()

    with tile.TileContext(nc) as tc:
        with (
            tc.tile_pool(name="persist", bufs=1) as persist,
            tc.tile_pool(name="w2s", bufs=3) as w2s,
            tc.tile_pool(name="ostage", bufs=4) as ostage,
            tc.tile_pool(name="psum", bufs=8, space="PSUM") as psum,
        ):
          with (tc.For_i(0, reps, 1) if reps > 1
                else __import__("contextlib").nullcontext()):
            # resident hm^T tiles: 8 full-row 1 MB DMAs (max DMA efficiency;
            # chunked early-start variants measured slower overall)
            hm = [persist.tile([128, N], bf, tag=f"hm{j}", name=f"hm{j}")
                  for j in range(JT)]
            for j in range(JT):
                nc.sync.dma_start(hm[j][:],
                                  hmt_d[j * 128:(j + 1) * 128, :])

            w2g = None
            for vb in range(VB):
                g, b = divmod(vb, 4)
                if b == 0:
                    w2g = w2s.tile([128, 4096], bf, tag="w2g")
                    nc.sync.dma_start(w2g[:],
                                      w2p_d[g * 128:(g + 1) * 128, :])
                wbase = b * 1024
                pss = [psum.tile([128, 512], f32, tag="ps", name=f"ps{vb}_{n}")
                       for n in range(NCH)]
                # j-outer: stationary switches once per 8 matmuls
                for j in range(JT):
                    for n in range(NCH):
                        nc.tensor.matmul(
                            pss[n][:],
                            w2g[:, wbase + j * 128:wbase + (j + 1) * 128],
                            hm[j][:, n * 512:(n + 1) * 512],
                            start=(j == 0), stop=(j == JT - 1),
                        )
                # evict fp32 PSUM -> fp16 SBUF on both DVE and ACT, then one
                # 512 KB DMA per 4 banks (out-DMA ring choice measured
                # neutral: sync/scalar/gpsimd within noise)
                for half in range(2):
                    ot = ostage.tile([128, 2048], f16, tag="ot")
                    for q in range(4):
                        n = half * 4 + q
                        dst = ot[:, q * 512:(q + 1) * 512]
                        if q % 2 == 0:
                            nc.vector.tensor_copy(dst, pss[n][:])
                        else:
                            nc.scalar.copy(dst, pss[n][:])
                    nc.sync.dma_start(
                        out_d[vb * 128:(vb + 1) * 128,
                              half * 2048:(half + 1) * 2048],
                        ot[:])

    nc.compile()
    return nc


def _routing_host(x, embed, expert_mu, expert_charge):
    """fp32 host replica of the routing math (same underflow semantics as
    the jax fp32 reference).  Returns (top_idx, top_w, h)."""
    h = embed[x.reshape(-1)].astype(np.float32)                    # (N, D)
    sq = (
        np.sum(h * h, axis=1, keepdims=True)
        + np.sum(expert_mu * expert_mu, axis=1)[None, :]
        - 2.0 * (h @ expert_mu.T)
    ).astype(np.float32)
    kern = np.exp(-np.maximum(sq, 0.0) / np.float32(2.0 * SIGMA ** 2),
                  dtype=np.float32)
    scores = kern * expert_charge[None, :].astype(np.float32)
    mean = scores.mean(axis=0, dtype=np.float32)
    # jax.lax.top_k: descending by value, ties broken by lower index
    top_idx = np.lexsort((np.arange(mean.shape[0]), -mean))[:TOP_K]
    return top_idx, scores[:, top_idx], h


def prepare_inputs(x, embed, fast_w1, fast_w2, expert_mu, expert_w,
                   expert_charge):
    """Host-side shard prep. Returns per-core input maps."""
    x = np.asarray(x).astype(np.int64).reshape(-1)
    embed = np.asarray(embed, dtype=np.float32)
    fast_w1 = np.asarray(fast_w1, dtype=np.float32)
    fast_w2 = np.asarray(fast_w2, dtype=np.float32)
    expert_mu = np.asarray(expert_mu, dtype=np.float32)
    expert_charge = np.asarray(expert_charge, dtype=np.float32)

    top_idx, top_w, h = _routing_host(x, embed, expert_mu, expert_charge)

    hm = np.maximum(h @ fast_w1.T, 0.0).astype(np.float32)
    if np.any(top_w):  # pragma: no cover - degenerate-input safety net
        expert_w = np.asarray(expert_w, dtype=np.float32)
        slow = np.zeros_like(hm)
        for k in range(TOP_K):
            slow += top_w[:, k:k + 1] * (h @ expert_w[top_idx[k]].T)
        hm = hm + np.float32(1.0 - FAST_RATIO) * slow
    hmt = np.ascontiguousarray(hm.T).astype(BF16)                  # (D, N)

    w2tb = fast_w2.T.astype(BF16)                                  # (D, V)
    w2t_full = np.zeros((D, VG * 4 * 128 * N_CORES), dtype=BF16)
    w2t_full[:, :V] = w2tb

    in_maps = []
    for c in range(N_CORES):
        sh = w2t_full[:, :V][:, c * VS:(c + 1) * VS]
        shp = np.zeros((D, VG * 4 * 128), dtype=BF16)
        shp[:, :sh.shape[1]] = sh
        # [j, p, g, b, c] -> [g, p, b, j, c] -> (VG*128, 4096)
        w2p = np.ascontiguousarray(
            shp.reshape(JT, 128, VG, 4, 128).transpose(2, 1, 3, 0, 4)
        ).reshape(VG * 128, 4096)
        in_maps.append({"ht": hmt, "w2p": w2p})
    return in_maps


def kernel(**inputs) -> np.ndarray:
    in_maps = prepare_inputs(**inputs)
    if "prog" not in _prog_cache:
        _prog_cache["prog"] = build_program()
    nc = _prog_cache["prog"]
    res = run_bass_kernel_spmd(nc, in_maps, core_ids=list(range(N_CORES)))
    # per-core output is transposed fp16 logits (VB*128, N); trim pad,
    # stack, transpose, upcast
    shards = [res.results[c]["out"][:VS] for c in range(N_CORES)]
    full_t = np.concatenate(shards, axis=0)[:V]      # (V, N) fp16
    return np.ascontiguousarray(full_t.T.astype(np.float32))


# revision 6
# speedup vs baseline: 1.1080x; 1.1080x over previous
"""Trainium2 Bass kernel for nn_HEMoETorch_43722767073393 (moe_routing).

Numerical structure exploited: with D=1024, ||h - mu||^2 ~ 1280 +- 60 for
every (token, expert) pair, so the RBF gate exp(-sq/8) < 1e-55 underflows
to exactly 0.0 in the fp32 reference for ALL pairs -> the expert (slow)
branch contributes exactly nothing.  The host verifies this with the same
fp32 underflow semantics (and falls back to adding the slow term on host
if it ever didn't hold), computes h_merged = relu(embed[x] @ w1^T) in
fp32, and ships h_merged^T in bf16.

Device (8 NeuronCores, vocab-sharded, no collectives): per core
    out^T[vs_block, tokens] = W2_shard[6400, 1024] @ hm^T   (52.9 GF bf16)
as 50 vocab blocks x (8 contraction x 8 token-chunk) matmuls accumulating
into all 8 PSUM banks, stationary operand switching once per 8 matmuls.

HW-measured design notes (For_i-slope method, same-process A/B):
  - w2 streamed as 1 MB group DMAs (4 vocab blocks, 3-deep prefetch);
    in-flight DMA writes into SBUF interfere with PE streaming, so fewer
    and bigger input transfers win.
  - hm^T as 8 full-row 1 MB DMAs: chunked early-start variants measured
    slower overall.
  - PSUM evictions all on DVE (keeping ACT off the PSUM path: -22 us)
    staging fp32 -> fp16 (halves out bytes; logits max ~2.1 so fp16
    rounding is ~1e-4 relative; host upcasts), one 1 MB out DMA per
    vocab block (-14 us vs 2x 512 KB).
  - Dead ends: fp8 DoubleRow (e4m3 quantization alone is 2.7-4.1e-2 rel
    err vs the 2e-2 gate), LDWEIGHTS dedup (walrus pairs each matmul
    with its own ldweights; deleting them breaks numerics, and ldweights
    count doesn't affect time), out-DMA issue-ring choice (neutral).
"""

import numpy as np
import ml_dtypes

import concourse.bass as bass  # noqa: F401  (bass must import before bacc)
import concourse.mybir as mybir
import concourse.tile as tile
from concourse import bacc
from concourse.bass_utils import run_bass_kernel_spmd

BF16 = ml_dtypes.bfloat16

N_CORES = 8
B, S = 4, 1024
N = B * S            # 4096 tokens
D = 1024
V = 50257
VS = 6283            # ceil(V / 8); padded total = 50264
JT = D // 128        # 8 contraction tiles
NCH = N // 512       # 8 token chunks (one PSUM bank each)
VB = 50              # ceil(VS/128): 128-wide vocab blocks (padded to 6400)
VG = (VB + 3) // 4   # 13 groups of 4 vocab blocks (1 MB DMA each)
SIGMA = 2.0
FAST_RATIO = 0.7
TOP_K = 8

_prog_cache: dict = {}


def build_program(N=N, D=D, VB=VB, num_devices=N_CORES, reps: int = 1):
    """Per-core SPMD program: out^T = W2_shard @ hm^T (bf16, fp16 out)."""
    JT = D // 128
    VG = (VB + 3) // 4
    nc = bacc.Bacc("TRN2", target_bir_lowering=False, debug=False,
                   num_devices=num_devices)
    bf = mybir.dt.bfloat16
    f16 = mybir.dt.float16
    f32 = mybir.dt.float32

    hmt_d = nc.dram_tensor("ht", [D, N], bf, kind="ExternalInput").ap()
    # pre-grouped on host: w2p4[g*128+p, b*1024 + j*128 + c]
    #   = w2T[j*128+p, (4g+b)*128 + c]
    w2p_d = nc.dram_tensor("w2p", [VG * 128, 4096], bf,
                           kind="ExternalInput").ap()
    out_d = nc.dram_tensor("out", [VB * 128, N], f16, kind="ExternalOutput").ap()

    with tile.TileContext(nc) as tc:
        with (
            tc.tile_pool(name="persist", bufs=1) as persist,
            tc.tile_pool(name="w2s", bufs=3) as w2s,
            tc.tile_pool(name="ostage", bufs=4) as ostage,
            tc.tile_pool(name="psum", bufs=8, space="PSUM") as psum,
        ):
          with (tc.For_i(0, reps, 1) if reps > 1
                else __import__("contextlib").nullcontext()):
            # resident hm^T tiles: 8 full-row 1 MB DMAs (max DMA efficiency;
            # chunked early-start variants measured slower overall)
            hm = [persist.tile([128, N], bf, tag=f"hm{j}", name=f"hm{j}")
                  for j in range(JT)]
            for j in range(JT):
                nc.sync.dma_start(hm[j][:],
                                  hmt_d[j * 128:(j + 1) * 128, :])

            w2g = None
            for vb in range(VB):
                g, b = divmod(vb, 4)
                if b == 0:
                    w2g = w2s.tile([128, 4096], bf, tag="w2g")
                    nc.sync.dma_start(w2g[:],
                                      w2p_d[g * 128:(g + 1) * 128, :])
                wbase = b * 1024
                pss = [psum.tile([128, 512], f32, tag="ps", name=f"ps{vb}_{n}")
                       for n in range(NCH)]
                # j-outer: stationary switches once per 8 matmuls
                for j in range(JT):
                    for n in range(NCH):
                        nc.tensor.matmul(
                            pss[n][:],
                            w2g[:, wbase + j * 128:wbase + (j + 1) * 128],
                            hm[j][:, n * 512:(n + 1) * 512],
                            start=(j == 0), stop=(j == JT - 1),
                        )
                # evict fp32 PSUM -> fp16 SBUF, all on DVE (keeping ACT off
                # the PSUM path measured -22us vs a DVE/ACT split), then one
                # 1 MB DMA per vocab block (-14us vs 2x 512KB)
                ot = ostage.tile([128, 4096], f16, tag="ot")
                for n in range(NCH):
                    nc.vector.tensor_copy(ot[:, n * 512:(n + 1) * 512],
                                          pss[n][:])
                nc.sync.dma_start(out_d[vb * 128:(vb + 1) * 128, :], ot[:])

    nc.compile()
    return nc


def _routing_host(x, embed, expert_mu, expert_charge):
    """fp32 host replica of the routing math (same underflow semantics as
    the jax fp32 reference).  Returns (top_idx, top_w, h)."""
    h = embed[x.reshape(-1)].astype(np.float32)                    # (N, D)
    sq = (
        np.sum(h * h, axis=1, keepdims=True)
        + np.sum(expert_mu * expert_mu, axis=1)[None, :]
        - 2.0 * (h @ expert_mu.T)
    ).astype(np.float32)
    kern = np.exp(-np.maximum(sq, 0.0) / np.float32(2.0 * SIGMA ** 2),
                  dtype=np.float32)
    scores = kern * expert_charge[None, :].astype(np.float32)
    mean = scores.mean(axis=0, dtype=np.float32)
    # jax.lax.top_k: descending by value, ties broken by lower index
    top_idx = np.lexsort((np.arange(mean.shape[0]), -mean))[:TOP_K]
    return top_idx, scores[:, top_idx], h


def prepare_inputs(x, embed, fast_w1, fast_w2, expert_mu, expert_w,
                   expert_charge):
    """Host-side shard prep. Returns per-core input maps."""
    x = np.asarray(x).astype(np.int64).reshape(-1)
    embed = np.asarray(embed, dtype=np.float32)
    fast_w1 = np.asarray(fast_w1, dtype=np.float32)
    fast_w2 = np.asarray(fast_w2, dtype=np.float32)
    expert_mu = np.asarray(expert_mu, dtype=np.float32)
    expert_charge = np.asarray(expert_charge, dtype=np.float32)

    top_idx, top_w, h = _routing_host(x, embed, expert_mu, expert_charge)

    hm = np.maximum(h @ fast_w1.T, 0.0).astype(np.float32)
    if np.any(top_w):  # pragma: no cover - degenerate-input safety net
        expert_w = np.asarray(expert_w, dtype=np.float32)
        slow = np.zeros_like(hm)
        for k in range(TOP_K):
            slow += top_w[:, k:k + 1] * (h @ expert_w[top_idx[k]].T)
        hm = hm + np.float32(1.0 - FAST_RATIO) * slow
    hmt = np.ascontiguousarray(hm.T).astype(BF16)                  # (D, N)

    w2tb = fast_w2.T.astype(BF16)                                  # (D, V)
    w2t_full = np.zeros((D, VG * 4 * 128 * N_CORES), dtype=BF16)
    w2t_full[:, :V] = w2tb

    in_maps = []
    for c in range(N_CORES):
        sh = w2t_full[:, :V][:, c * VS:(c + 1) * VS]
        shp = np.zeros((D, VG * 4 * 128), dtype=BF16)
        shp[:, :sh.shape[1]] = sh
        # [j, p, g, b, c] -> [g, p, b, j, c] -> (VG*128, 4096)
        w2p = np.ascontiguousarray(
            shp.reshape(JT, 128, VG, 4, 128).transpose(2, 1, 3, 0, 4)
        ).reshape(VG * 128, 4096)
        in_maps.append({"ht": hmt, "w2p": w2p})
    return in_maps


def kernel(**inputs) -> np.ndarray:
    in_maps = prepare_inputs(**inputs)
    if "prog" not in _prog_cache:
        _prog_cache["prog"] = build_program()
    nc = _prog_cache["prog"]
    res = run_bass_kernel_spmd(nc, in_maps, core_ids=list(range(N_CORES)))
    # per-core output is transposed fp16 logits (VB*128, N); trim pad,
    # stack, transpose, upcast
    shards = [res.results[c]["out"][:VS] for c in range(N_CORES)]
    full_t = np.concatenate(shards, axis=0)[:V]      # (V, N) fp16
    return np.ascontiguousarray(full_t.T.astype(np.float32))
